# revision 3
# baseline (speedup 1.0000x reference)
"""ActionEncoder Trainium2 kernel (8 NeuronCores, expert-parallel).

Strategy:
- Host groups the 32768 flat actions by action_type (1=pick, 2=transport,
  3=move; type 0 rows are wait_emb and never touch the device), splits each
  group evenly across the 8 cores, and pads each per-core shard to a
  multiple of 128 (pad rows gather table row 0 and are discarded).
- Each core: dma_gather (SWDGE, 4 queues) pulls embedding rows action-major,
  PE transposes them to feature-major, then two fp32r GEMMs per expert with
  the LeakyReLU(0.01)+bias fused into ScalarE Prelu. Outputs are written
  feature-major [256, C] and un-transposed/scattered on the host.
- Weights/tables are replicated per core; everything runs as one SPMD NEFF.
"""
import sys

import numpy as np

sys.path.insert(0, "/opt/trn_rl_repo")

import concourse.bass as bass
import concourse.bacc as bacc
import concourse.mybir as mybir
import concourse.tile as tile
from concourse import library_config
from concourse.bass_utils import run_bass_kernel_spmd

D = 256
HID = 512
OUT = 256
NTAB = 8192
NCORES = 8
NA = 384  # actions per compute chunk (matmul moving dim)
FP32 = mybir.dt.float32
FP32R = mybir.dt.float32r
INT16 = mybir.dt.int16

LAST_RESULT = None  # BassKernelResults of the most recent kernel() call

# (name, number of gathered tables, layer-1 K)
EXPERTS = (
    ("pick", ("agv", "from", "to", "mach"), 4 * D),
    ("trans", ("agv", "mach"), 2 * D),
    ("move", ("agv", "mach"), 2 * D),
)


def _chunks(c):
    """Split capacity c into chunks of <=NA, each a multiple of 128."""
    out = []
    pos = 0
    while pos < c:
        n = min(NA, c - pos)
        out.append((pos, n))
        pos += n
    return out


def _build(caps):
    """Emit the per-core BIR. caps = dict expert -> padded capacity."""
    nc = bacc.Bacc(num_swdge_queues=4)

    tabs = {
        "emb_operation": nc.declare_dram_parameter("emb_operation", [NTAB, D], FP32R, isOutput=False),
        "emb_machine": nc.declare_dram_parameter("emb_machine", [NTAB, D], FP32R, isOutput=False),
        "emb_AGV": nc.declare_dram_parameter("emb_AGV", [NTAB, D], FP32R, isOutput=False),
    }
    table_of = {"agv": "emb_AGV", "from": "emb_operation", "to": "emb_operation", "mach": "emb_machine"}

    params = {}
    for name, tables, K in EXPERTS:
        c = caps[name]
        if c == 0:
            continue
        params[f"{name}_W1"] = nc.declare_dram_parameter(f"{name}_W1", [128, K // 128, HID], FP32R, isOutput=False)
        params[f"{name}_W2"] = nc.declare_dram_parameter(f"{name}_W2", [128, HID // 128, OUT], FP32R, isOutput=False)
        params[f"{name}_b1"] = nc.declare_dram_parameter(f"{name}_b1", [128, HID // 128], FP32, isOutput=False)
        params[f"{name}_b2"] = nc.declare_dram_parameter(f"{name}_b2", [128, OUT // 128], FP32, isOutput=False)
        for t in tables:
            params[f"{name}_idx_{t}"] = nc.declare_dram_parameter(f"{name}_idx_{t}", [128, c // 16], INT16, isOutput=False)
        params[f"{name}_outT"] = nc.declare_dram_parameter(f"{name}_outT", [OUT, c], FP32, isOutput=True)
    params["ident_in"] = nc.declare_dram_parameter("ident_in", [128, 128], FP32R, isOutput=False)

    qrr = [0]  # SWDGE queue round-robin counter

    with tile.TileContext(nc) as tc:
        with (
            tc.tile_pool(name="wp", bufs=1) as wp,
            tc.tile_pool(name="xp", bufs=2) as xp,
            tc.tile_pool(name="ps", bufs=1, space="PSUM") as ps,
        ):
            nc.gpsimd.load_library(library_config.mlp)

            ident = wp.tile([128, 128], FP32R, name="ident")
            nc.sync.dma_start(out=ident[:], in_=params["ident_in"][:])

            # per-expert static tiles
            W1 = {}
            W2 = {}
            B1 = {}
            B2 = {}
            IDX = {}
            for name, tables, K in EXPERTS:
                c = caps[name]
                if c == 0:
                    continue
                W1[name] = wp.tile([128, K // 128, HID], FP32R, name=f"w1_{name}")
                nc.sync.dma_start(out=W1[name][:], in_=params[f"{name}_W1"][:])
                W2[name] = wp.tile([128, HID // 128, OUT], FP32R, name=f"w2_{name}")
                nc.sync.dma_start(out=W2[name][:], in_=params[f"{name}_W2"][:])
                B1[name] = wp.tile([128, HID // 128], FP32, name=f"b1_{name}")
                nc.sync.dma_start(out=B1[name][:], in_=params[f"{name}_b1"][:])
                B2[name] = wp.tile([128, OUT // 128], FP32, name=f"b2_{name}")
                nc.sync.dma_start(out=B2[name][:], in_=params[f"{name}_b2"][:])
                for t in tables:
                    it = wp.tile([128, c // 16], INT16, name=f"idx_{name}_{t}")
                    nc.sync.dma_start(out=it[:], in_=params[f"{name}_idx_{t}"][:])
                    IDX[(name, t)] = it

            for name, tables, K in EXPERTS:
                c = caps[name]
                if c == 0:
                    continue
                for pos, n in _chunks(c):
                    nb = n // 128
                    # gather this chunk's rows, one call per table
                    grp = "pick" if name == "pick" else "tm"
                    xc = {}
                    for t in tables:
                        g = xp.tile([128, nb, D], FP32R, tag=f"g_{grp}_{t}", name=f"g_{name}_{t}")
                        nc.gpsimd.dma_gather(
                            g[:],
                            tabs[table_of[t]][:],
                            IDX[(name, t)][:, pos // 16 : (pos + n) // 16],
                            n,
                            n,
                            D,
                            queue_num=qrr[0] % 4,
                        )
                        qrr[0] += 1
                        xc[t] = g

                    # transpose to feature-major XT [128, K/128, n]
                    xT = xp.tile([128, K // 128, n], FP32R, tag=f"xT_{grp}", name=f"xT_{name}")
                    for kd in range(K // 128):
                        t = tables[kd // 2]
                        h = kd % 2
                        for s in range(nb):
                            pt = ps.tile([128, 128], FP32R, space="PSUM", tag="pt", bufs=2, name="pt")
                            nc.tensor.transpose(
                                out=pt[:],
                                in_=xc[t][:, s, h * 128 : (h + 1) * 128],
                                identity=ident[:],
                            )
                            nc.vector.tensor_copy(out=xT[:, kd, s * 128 : (s + 1) * 128], in_=pt[:])

                    # layer 1: H = Prelu(X @ W1 + b1), feature-major
                    hT = xp.tile([128, HID // 128, n], FP32R, tag="hT", name=f"hT_{name}")
                    for m in range(HID // 128):
                        p1 = ps.tile([128, NA], FP32, space="PSUM", tag="p1", bufs=2, name="p1")
                        for k in range(K // 128):
                            nc.tensor.matmul(
                                out=p1[:, :n],
                                lhsT=W1[name][:, k, m * 128 : (m + 1) * 128],
                                rhs=xT[:, k, :],
                                start=(k == 0),
                                stop=(k == K // 128 - 1),
                            )
                        nc.scalar.activation(
                            out=hT[:, m, :],
                            in_=p1[:, :n],
                            func=mybir.ActivationFunctionType.Prelu,
                            bias=B1[name][:, m : m + 1],
                            scale=1.0,
                            alpha=0.01,
                        )

                    # layer 2: O = H @ W2 + b2, feature-major
                    osb = xp.tile([128, OUT // 128, n], FP32, tag="o", name=f"o_{name}")
                    for m2 in range(OUT // 128):
                        p2 = ps.tile([128, NA], FP32, space="PSUM", tag="p2", bufs=2, name="p2")
                        for k2 in range(HID // 128):
                            nc.tensor.matmul(
                                out=p2[:, :n],
                                lhsT=W2[name][:, k2, m2 * 128 : (m2 + 1) * 128],
                                rhs=hT[:, k2, :],
                                start=(k2 == 0),
                                stop=(k2 == HID // 128 - 1),
                            )
                        nc.scalar.activation(
                            out=osb[:, m2, :],
                            in_=p2[:, :n],
                            func=mybir.ActivationFunctionType.Identity,
                            bias=B2[name][:, m2 : m2 + 1],
                            scale=1.0,
                        )
                    for m2 in range(OUT // 128):
                        nc.sync.dma_start(
                            out=params[f"{name}_outT"][m2 * 128 : (m2 + 1) * 128, pos : pos + n],
                            in_=osb[:, m2, :],
                        )

    nc.finalize()
    return nc


def _wrap_idx(idx, c):
    """int array [c] -> wrapped int16 [128, c//16] for dma_gather."""
    w = idx.astype(np.int16).reshape(c // 16, 16).T
    return np.ascontiguousarray(np.tile(w, (8, 1)))


def _prep_w1(w1):
    """[K, HID] -> [128, K//128, HID]"""
    k = w1.shape[0]
    return np.ascontiguousarray(w1.reshape(k // 128, 128, -1).transpose(1, 0, 2))


def _prep_b(b):
    """[n] -> [128, n//128]"""
    return np.ascontiguousarray(b.reshape(-1, 128).T)


def kernel(**inputs):
    at = np.asarray(inputs["action_type"])
    n_act = at.shape[0]
    out = np.empty((n_act, OUT), dtype=np.float32)

    idx_in = {
        "agv": np.asarray(inputs["agv_idx"]),
        "from": np.asarray(inputs["op_from_idx"]),
        "to": np.asarray(inputs["op_to_idx"]),
        "mach": np.asarray(inputs["machine_idx"]),
    }

    rows = {}
    caps = {}
    pers = {}
    for tcode, (name, tables, K) in zip((1, 2, 3), EXPERTS):
        r = np.nonzero(at == tcode)[0]
        rows[name] = r
        pers[name] = -(-max(len(r), 1) // NCORES)  # ceil, >=1
        caps[name] = -(-pers[name] // 128) * 128

    nc = _build(caps)

    in_maps = []
    for core in range(NCORES):
        m = {
            "emb_operation": np.asarray(inputs["emb_operation"]),
            "emb_machine": np.asarray(inputs["emb_machine"]),
            "emb_AGV": np.asarray(inputs["emb_AGV"]),
            "ident_in": np.eye(128, dtype=np.float32),
        }
        for name, tables, K in EXPERTS:
            c = caps[name]
            if c == 0:
                continue
            m[f"{name}_W1"] = _prep_w1(np.asarray(inputs[f"{name}_W1"]))
            m[f"{name}_W2"] = _prep_w1(np.asarray(inputs[f"{name}_W2"]))
            m[f"{name}_b1"] = _prep_b(np.asarray(inputs[f"{name}_b1"]))
            m[f"{name}_b2"] = _prep_b(np.asarray(inputs[f"{name}_b2"]))
            r = rows[name]
            per = pers[name]
            shard = r[core * per : (core + 1) * per]
            pad = np.zeros(c, dtype=np.int64)
            pad[: len(shard)] = shard
            for t in tables:
                m[f"{name}_idx_{t}"] = _wrap_idx(idx_in[t][pad], c)
        in_maps.append(m)

    res = run_bass_kernel_spmd(nc, in_maps, list(range(NCORES)))
    global LAST_RESULT
    LAST_RESULT = res

    # assemble
    wait_rows = np.nonzero(at == 0)[0]
    out[wait_rows] = np.asarray(inputs["wait_emb"])[None, :].astype(np.float32)
    for name, tables, K in EXPERTS:
        c = caps[name]
        r = rows[name]
        if len(r) == 0:
            continue
        per = pers[name]
        full = np.concatenate(
            [res.results[core][f"{name}_outT"].T[:per] for core in range(NCORES)],
            axis=0,
        )
        out[r] = full[: len(r)]
    return out


